# revision 66
# baseline (speedup 1.0000x reference)
"""Trainium2 Bass kernel for nn_Attention_40785009443452.

Reference computation (per batch b):
    qkv = w_qkv @ x_b            # 1x1x1 conv == channel linear
    q,k,v split into 4 heads of dim 16, tokens N = 16*16*16 = 4096
    q,k L2-normalized along head dim
    attn = softmax(q @ k^T)      # [N, N] per (b, head)
    out  = attn @ v  (+ x residual)

Sharding: 8 (batch, head) pairs -> 8 NeuronCores (data + head parallel).
Each core computes one full 4096x4096 attention.

Device algorithm (per core), S^T orientation so softmax reduction (over
keys) lands on the PSUM partition axis and is folded into the PV matmul
via an appended ones-column on V:

    B  = Wq^T Wk                     [64, 64]   (tiny matmul on device)
    G' = (B^T X) * rq  (col scale)   rq[n] = 1/||q_n||
    X' = X * rk                      rk[m] = 1/||k_m||
    S^T chunk [128 keys, 1024 qry] = X'^T(keys) @ G'(cols)  2x concurrent
                                     K=64 matmuls on PE row groups 0/64
    P^T = exp(S^T)                   2 of 3 chunks on ACT; every 3rd on
                                     the DVE via a custom quartic-poly op
                                     (EXP4_ANT; softmax is scale-invariant
                                     and tolerates its ~1.6e-3 rel err)
    O'a [33, 512]  += V'_j^T P^T     queries 0-511,   PE col group 0
    O'b [33, 512]  += V'_j^T P^T     queries 512-1023, PE col group 64
                                     (concurrent col-split PV matmuls)
    out^T = O'[0:16] / O'[32] + x_res

Main-loop PSUM: three S buffers (2 banks each, SEPARATE tiles = separate
conflict domains) + two PV accumulators = exactly 8 banks.  Emission is
software-pipelined with a 2-group S lookahead: the PE queue is strict
in-order, so exp-dependent PV matmuls must never sit ahead of the
independent next S matmuls (head-of-line blocking stalls ACT and lets
HAM re-throttle the PE).  Concurrent same-row-group matmul pairs must
drain to DIFFERENT PSUM banks (same-bank concurrent PE drains are a
fatal collision).

Normalization scales via exp(-0.5*ln(sumsq)) on ACT (Rsqrt/Reciprocal
activations banned for accuracy; Ln+Exp live in one ACT table set).

Measured: 176-180us HW exec (baseline 215us); rel err 5.8e-5.
"""

import os

import numpy as np

import concourse.bass as bass
import concourse.mybir as mybir
import concourse.tile as tile
from concourse import bacc
from concourse.bass_utils import run_bass_kernel_spmd

NCORES = 8
C = 64          # channels
HEADS = 4
HD = 16         # head dim
N = 4096        # tokens (16*16*16)
NBQ = 1024      # queries per block
NB = N // NBQ   # 4 blocks
KC = 128        # keys per chunk
JT = N // KC    # 32 key chunks
FP = mybir.dt.float32
BF = mybir.dt.bfloat16

AF = mybir.ActivationFunctionType

# env kill-switches for risky features
K_PAIR = os.environ.get("K_PAIR", "0") == "1"       # paired-buffer exp
K_PVSPLIT = os.environ.get("K_PVSPLIT", "1") == "1"  # col-group PV split
K_GPMUL = os.environ.get("K_GPMUL", "1") == "1"      # X*rk mul on GPSIMD
K_VREC = os.environ.get("K_VREC", "0") == "1"        # DVE reciprocal epilogue
K_DVEEXP = os.environ.get("K_DVEEXP", "1") == "1"    # singles' exp on DVE
# route every Nth pair's exp to DVE too (0 = none)
K_DVEPAIRS = int(os.environ.get("K_DVEPAIRS", "0"))

# Quartic exp for the DVE (softmax tolerates the ~1.6e-3 rel err):
#   p(x) = 1 + x*(B1 + x*(B2 + x*(B3 + B4*x)))  ~=  exp(x) on [-1.06, 1.06]
# fits the DVE's 8 ALU stages exactly (4 mult + 4 add); B4 rides the
# spilled-C3 slot (a [P,1] tensor read once at element 0).
EB1, EB2, EB3, EB4 = 0.99985291, 0.50492711, 0.17330073, 0.03599347


def _register_exp4():
    from concourse import dve_ops as _dve_ops
    from concourse.dve_spec import (
        Spec, Src0, C0, C1, C2, C3, One, lower, _spill_c3_to_src1, _has_src1,
    )
    from concourse.dve_uop import DveOpSpec

    for op in _dve_ops.OPS:
        if op.name == "EXP4_ANT":
            return op
    x = Src0
    body = _spill_c3_to_src1(One + x * (C0 + x * (C1 + x * (C2 + x * C3))))
    spec = Spec(
        body=body,
        reference=lambda in0, in1, s0, s1, imm2: 1
        + in0 * (s0 + in0 * (s1 + in0 * (imm2 + in0 * in1))),
    )
    shas = {}
    for ver in ("v3", "v4"):
        t = DveOpSpec(name="EXP4_ANT", opcode=0, uops=lower(spec, ver=ver),
                      rd1_en=_has_src1(spec))
        shas[ver] = t.sha(ver)
    op = _dve_ops.DveOp("EXP4_ANT", spec, subdim=False, uops_sha=shas)
    _register_op(op)
    return op


def _register_op(op):
    from concourse import dve_ops as _dve_ops

    _dve_ops.OPS.append(op)
    # the registry dicts are comprehensions over OPS at module import time
    _dve_ops.CUSTOM_DVE_SPECS[op.name] = op.spec
    row = _dve_ops._CUSTOM_DVE_ROW_BASE + len(_dve_ops.OPS) - 1
    assert row < 0x20, "custom DVE row field overflow"
    _dve_ops._SUB_OPCODE_FOR_NAME[op.name] = row


def _register_sq():
    """x^2 with a SINGLE tensor stream, so the input may live in PSUM
    (stock tensor_tensor(x, x) needs two read ports = SBUF only)."""
    from concourse import dve_ops as _dve_ops
    from concourse.dve_spec import Spec, Src0, lower, sq, _has_src1
    from concourse.dve_uop import DveOpSpec

    for op in _dve_ops.OPS:
        if op.name == "SQ1_ANT":
            return op
    spec = Spec(body=sq(Src0), reference=lambda in0: in0 * in0)
    shas = {}
    for ver in ("v3", "v4"):
        t = DveOpSpec(name="SQ1_ANT", opcode=0, uops=lower(spec, ver=ver),
                      rd1_en=_has_src1(spec))
        shas[ver] = t.sha(ver)
    op = _dve_ops.DveOp("SQ1_ANT", spec, subdim=False, uops_sha=shas)
    _register_op(op)
    return op


EXP4 = _register_exp4()
SQ1 = _register_sq()


def build_program():
    nc = bacc.Bacc(
        "TRN2", target_bir_lowering=False, debug=False, enable_asserts=False
    )
    x_d = nc.dram_tensor("x", [C, N], FP, kind="ExternalInput").ap()
    w_d = nc.dram_tensor("w", [3 * HD, C], FP, kind="ExternalInput").ap()
    wT_d = nc.dram_tensor("wT", [C, 3 * HD], FP, kind="ExternalInput").ap()
    xr_d = nc.dram_tensor("xres", [HD, N], FP, kind="ExternalInput").ap()
    op_d = nc.dram_tensor("onespat", [2 * HD, 33], FP,
                          kind="ExternalInput").ap()
    out_d = nc.dram_tensor("out", [HD, N], FP, kind="ExternalOutput").ap()

    with tile.TileContext(nc) as tc:
        _body(tc, x_d, w_d, wT_d, xr_d, op_d, out_d)
    nc.compile()
    return nc


def _body(tc, x_d, w_d, wT_d, xr_d, op_d, out_d):
    nc = tc.nc
    import contextlib

    # Pre-load the ACT table set containing Exp, Ln AND Square so the
    # compiler's per-function chooser doesn't flip-flop between sets.
    if os.environ.get("K_PRELOAD", "1") == "1":
        from concourse.hw_specs import get_activation_tables

        set_names = list(get_activation_tables(nc.m.arch).keys())
        set_id = set_names.index("natural_log_exp_and_others")
        nc.scalar.add_instruction(
            mybir.InstLoadActFuncSet(
                name=f"I-{nc.next_id()}", act_func_set_id=set_id
            )
        )

    with contextlib.ExitStack() as ctx:
        consts = ctx.enter_context(tc.tile_pool(name="consts", bufs=1))

        # ---- load inputs -------------------------------------------------
        # weights on the gpsimd DMA queue; X split across sync+gpsimd
        wq_eng = nc.gpsimd
        WT = consts.tile([C, 3 * HD], FP)
        wq_eng.dma_start(WT, wT_d)
        Wq = consts.tile([HD, C], FP)
        wq_eng.dma_start(Wq, w_d[0:HD, :])
        Wk = consts.tile([HD, C], FP)
        wq_eng.dma_start(Wk, w_d[HD : 2 * HD, :])

        ones1_16 = consts.tile([1, HD], BF)
        nc.any.memset(ones1_16, 1.0)
        ones1_16f = consts.tile([1, HD], FP)
        nc.any.memset(ones1_16f, 1.0)
        ones33 = consts.tile([33, 2 * C], BF)
        nc.any.memset(ones33, 1.0)
        eps33 = consts.tile([33, 1], FP)
        nc.any.memset(eps33, 1e-24)
        b4t = consts.tile([KC, 1], FP)
        nc.any.memset(b4t, EB4)


        # [wqk | wqk] duplicated along BOTH rows (PE row-group alternation
        # across prologue chunks) and cols (qk-proj lands on partitions
        # 0-31 or 32-63 so the sumsq matmul can alternate row strips too)
        WTqk4 = consts.tile([2 * C, 4 * HD], BF)
        for rh in range(2):
            for ch in range(2):
                nc.vector.tensor_copy(
                    WTqk4[rh * C : rh * C + C,
                          ch * 2 * HD : ch * 2 * HD + 2 * HD],
                    WT[:, 0 : 2 * HD])
        ones_pat_f2 = consts.tile([2 * HD, 33], FP)
        nc.sync.dma_start(ones_pat_f2, op_d)
        ones_pat2 = consts.tile([4 * HD, 33], BF)
        nc.vector.tensor_copy(ones_pat2[0 : 2 * HD, :], ones_pat_f2)
        nc.vector.tensor_copy(ones_pat2[2 * HD : 4 * HD, :], ones_pat_f2)
        WTv2 = consts.tile([2 * C, HD], BF)
        nc.vector.tensor_copy(WTv2[0:C, :], WT[:, 2 * HD : 3 * HD])
        nc.vector.tensor_copy(WTv2[C : 2 * C, :], WT[:, 2 * HD : 3 * HD])
        X = consts.tile([C, N], FP)
        Xb2 = consts.tile([2 * C, N], BF)      # bf16 X duplicated rows
        for c8 in range(8):
            sl = slice(c8 * 512, c8 * 512 + 512)
            nc.sync.dma_start(X[:, sl], x_d[:, sl])
            nc.vector.tensor_copy(Xb2[0:C, sl], X[:, sl])
            nc.scalar.copy(Xb2[C : 2 * C, sl], X[:, sl])
        XR = consts.tile([HD, N], FP)
        wq_eng.dma_start(XR, xr_d)

        Bsb2b = consts.tile([2 * C, 2 * C], BF)  # [B | B] dup rows, bf16
        Gp2 = consts.tile([2 * C, N], BF)      # (B^T X)*rq duplicated rows
        Xp2 = consts.tile([2 * C, N], BF)      # X*rk duplicated rows
        # [V_j(16) | zeros | ones@32] stationary tiles for the PV matmul;
        # ones column lands the softmax denominator on PSUM partition 32/96.
        Vp = consts.tile([KC, JT, 33], BF)
        nc.any.memset(Vp, 0.0)
        nc.any.memset(Vp[:, :, 32], 1.0)

        # ---- prologue: B, norms, G', X', V' tiles ------------------------
        # 512-wide chunks through a 6-deep 1-bank slot rotation: a chunk's
        # norm chain never waits on the previous chunk's tail, so the 8
        # chunks pipeline at the per-engine rate (ACT: Ln+Exp, DVE: sq +
        # rep cast + mulG, GPSIMD: X*rk mul, ACT: repk cast).
        with contextlib.ExitStack() as mctx:
            pps = mctx.enter_context(
                tc.tile_pool(name="prol_ps", bufs=6, space="PSUM"))
            psb = mctx.enter_context(tc.tile_pool(name="prol_sb", bufs=2))

            # B = Wq^T Wk (fp32), duplicated into [B | B] on both row halves
            ps_b = pps.tile([C, C], FP, tag="pp2", bufs=3)
            nc.tensor.matmul(ps_b, Wq, Wk, start=True, stop=True)
            for rh in range(2):
                for ch in range(2):
                    nc.vector.tensor_copy(
                        Bsb2b[rh * C : rh * C + C, ch * C : ch * C + C],
                        ps_b)

            for c4 in range(4):
                sl = slice(c4 * 1024, c4 * 1024 + 1024)
                ps_q = pps.tile([2 * HD, 1024], FP, tag="pp2", bufs=3)
                for h2 in range(2):
                    hsl = slice(h2 * 512, h2 * 512 + 512)
                    xsl = slice(c4 * 1024 + h2 * 512,
                                c4 * 1024 + h2 * 512 + 512)
                    rg = slice(h2 * C, h2 * C + C)
                    nc.tensor.matmul(ps_q[:, hsl], WTqk4[rg, 0 : 2 * HD],
                                     Xb2[rg, xsl], start=True, stop=True)
                sqq = psb.tile([2 * HD, 1024], BF, tag="sq")
                nc.scalar.activation(sqq, ps_q, AF.Square)
                ps_nq = pps.tile([33, 1024], FP, tag="pp2", bufs=3)
                for h2 in range(2):
                    hsl = slice(h2 * 512, h2 * 512 + 512)
                    nc.tensor.matmul(ps_nq[:, hsl], ones_pat2[0 : 2 * HD, :],
                                     sqq[:, hsl], start=True, stop=True)
                lnq = psb.tile([33, 1024], FP, tag="ln")
                nc.scalar.activation(lnq, ps_nq, AF.Ln, bias=eps33)
                rqk = psb.tile([33, 1024], BF, tag="rqk")
                nc.scalar.activation(rqk, lnq, AF.Exp, scale=-0.5)
                ps_rep = pps.tile([2 * C, 1024], FP, tag="pp2", bufs=3)
                ps_repk = pps.tile([2 * C, 1024], FP, tag="pp2", bufs=3)
                for h2 in range(2):
                    hsl = slice(h2 * 512, h2 * 512 + 512)
                    nc.tensor.matmul(ps_rep[:, hsl], ones33[0:1, :],
                                     rqk[0:1, hsl], start=True, stop=True)
                    nc.tensor.matmul(ps_repk[:, hsl], ones33[32:33, :],
                                     rqk[32:33, hsl], start=True, stop=True)
                rep_sb = psb.tile([2 * C, 1024], BF, tag="rep")
                nc.vector.tensor_copy(rep_sb, ps_rep)
                repk_sb = psb.tile([2 * C, 1024], BF, tag="repk")
                nc.vector.tensor_copy(repk_sb, ps_repk)
                ps_g = pps.tile([2 * C, 1024], FP, tag="pp2", bufs=3)
                for h2 in range(2):
                    hsl = slice(h2 * 512, h2 * 512 + 512)
                    xsl = slice(c4 * 1024 + h2 * 512,
                                c4 * 1024 + h2 * 512 + 512)
                    rg = slice(h2 * C, h2 * C + C)
                    nc.tensor.matmul(ps_g[:, hsl], Bsb2b[rg, :],
                                     Xb2[rg, xsl], start=True, stop=True)
                nc.vector.tensor_mul(Gp2[:, sl], ps_g, rep_sb)
                if K_GPMUL:
                    nc.gpsimd.tensor_mul(Xp2[:, sl], Xb2[:, sl], repk_sb)
                else:
                    nc.vector.tensor_mul(Xp2[0:C, sl], ps_repk[0:C, :],
                                         X[:, sl])
                    nc.vector.tensor_mul(Xp2[C : 2 * C, sl],
                                         ps_repk[C : 2 * C, :], X[:, sl])

                ps_kv8 = pps.tile([KC, 8, HD], FP, tag="ppv", bufs=2)
                for jj in range(8):
                    j = 8 * c4 + jj
                    ksl = slice(j * KC, j * KC + KC)
                    nc.tensor.matmul(ps_kv8[:, jj, :], Xb2[0:C, ksl],
                                     WTv2[0:C, :], start=True, stop=True)
                nc.vector.tensor_copy(
                    Vp[:, 8 * c4 : 8 * c4 + 8, 0:HD], ps_kv8)
        # ---- main attention loop ----------------------------------------
        with contextlib.ExitStack() as mctx:
            ps_pool = mctx.enter_context(
                tc.tile_pool(name="ps_main", bufs=1, space="PSUM"))
            pt_pool = mctx.enter_context(tc.tile_pool(name="pt", bufs=12))
            ep_pool = mctx.enter_context(tc.tile_pool(name="ep", bufs=2))

            # S^T triple buffer: pair slots as ONE flat [128, 2048] when
            # pairing (the pair-exp AP must be contiguous in one tile),
            # else three separate tiles (= separate PSUM conflict domains).
            if K_PAIR:
                ps_A = ps_pool.tile([KC, 2 * NBQ], FP, tag="ps_A")
                ps_B = ps_pool.tile([KC, NBQ], FP, tag="ps_B")
                slots = [(ps_A, 0), (ps_A, NBQ), (ps_B, 0)]
            else:
                ps_A0 = ps_pool.tile([KC, NBQ], FP, tag="ps_A0")
                ps_A1 = ps_pool.tile([KC, NBQ], FP, tag="ps_A1")
                ps_B = ps_pool.tile([KC, NBQ], FP, tag="ps_B")
                ps_A = None
                slots = [(ps_A0, 0), (ps_A1, 0), (ps_B, 0)]

            def epilogue(nb, po_a, po_b):
                nbase = nb * NBQ
                oall = ep_pool.tile([33, NBQ], FP, tag="oall",
                                    name=f"oall_{nb}")
                # Ln reads the denominator rows straight from PSUM (32- and
                # 96-aligned), in parallel with the oall evacuation
                lnd2 = ep_pool.tile([1, NBQ], FP, tag="lnd2",
                                    name=f"lnd2_{nb}")
                nc.scalar.activation(lnd2[:, 0:512], po_a[32:33, :], AF.Ln)
                nc.scalar.activation(lnd2[:, 512:1024], po_b[96:97, :],
                                     AF.Ln)
                nc.vector.tensor_copy(oall[:, 0:512], po_a)
                nc.vector.tensor_copy(oall[:, 512:1024], po_b[C:97, :])
                if K_VREC:
                    rinv = ep_pool.tile([1, NBQ], FP, tag="rinv",
                                        name=f"rinv_{nb}")
                    nc.vector.reciprocal(rinv, oall[32:33, :])
                    rep_lhs = ones1_16f
                else:
                    rinv = ep_pool.tile([1, NBQ], BF, tag="rinv",
                                        name=f"rinv_{nb}")
                    nc.scalar.activation(rinv, lnd2, AF.Exp, scale=-1.0)
                    rep_lhs = ones1_16
                t2 = ep_pool.tile([HD, NBQ], FP, tag="t2", name=f"t2_{nb}")
                for h2 in range(2):
                    qsl = slice(h2 * 512, h2 * 512 + 512)
                    tag = "po_a" if h2 == 0 else "po_b"
                    shape = [HD, 512] if h2 == 0 else [C + HD, 512]
                    ps_rep = ps_pool.tile(shape, FP, tag=tag,
                                          name=f"ps_rep_{nb}_{h2}")
                    pr = ps_rep if h2 == 0 else ps_rep[C : C + HD, :]
                    nc.tensor.matmul(pr, rep_lhs, rinv[:, qsl],
                                     start=True, stop=True)
                    nc.vector.tensor_mul(t2[:, qsl], oall[0:HD, qsl], pr)
                osb = ep_pool.tile([HD, NBQ], FP, tag="osb",
                                   name=f"osb_{nb}")
                osl = slice(nbase, nbase + NBQ)
                nc.vector.tensor_add(osb, t2, XR[:, osl])
                nc.sync.dma_start(out_d[:, osl], osb)

            def s_matmul(g):
                nb, j = divmod(g, JT)
                nbase = nb * NBQ
                t = g % 3
                ksl = slice(j * KC, j * KC + KC)
                stile, sbase = slots[t]
                for h2 in range(2):
                    gsl = slice(nbase + h2 * 512, nbase + h2 * 512 + 512)
                    rg = slice(h2 * C, h2 * C + C)  # alternate row groups
                    out = stile[:, sbase + h2 * 512 : sbase + h2 * 512 + 512]
                    nc.tensor.matmul(out, Xp2[rg, ksl],
                                     Gp2[rg, gsl], start=True, stop=True)

            po = {}

            def ensure_po(nb):
                if nb not in po:
                    po_a = ps_pool.tile([33, 512], FP, tag="po_a",
                                        name=f"po_a_{nb}")
                    po_b = ps_pool.tile([97, 512], FP, tag="po_b",
                                        name=f"po_b_{nb}")
                    po[nb] = (po_a, po_b)
                return po[nb]

            def pv_matmul(g, pt_ap):
                nb, j = divmod(g, JT)
                # PV accumulators are single-buffered: emit the previous
                # block's epilogue (its readers) before this block's first
                # PV write, even when an exp pair straddles two blocks.
                if j == 0 and nb > 0:
                    epilogue(nb - 1, *po[nb - 1])
                po_a, po_b = ensure_po(nb)
                first, last = j == 0, j == JT - 1
                nc.tensor.matmul(po_a, Vp[:, j, :], pt_ap[:, 0:512],
                                 start=first, stop=last)
                if K_PVSPLIT:
                    nc.tensor.matmul(po_b[C:97, :], Vp[:, j, :],
                                     pt_ap[:, 512:1024],
                                     start=first, stop=last)
                else:
                    nc.tensor.matmul(po_b[0:33, :], Vp[:, j, :],
                                     pt_ap[:, 512:1024],
                                     start=first, stop=last)

            # GLOBAL chunk index g over all blocks so the pair/single
            # pattern alternates strictly: P(0,1) S(2) P(0,1) S(2) ...
            # Slots (0,1) are consumed by paired exps (FD=2048), slot 2
            # by singles.  A pair may straddle two query blocks.
            CH = NB * JT
            groups = []
            g = 0
            while g < CH:
                if K_PAIR and g % 3 < 2 and g + 1 < CH:
                    groups.append((g, g + 1))
                    g += 2
                else:
                    groups.append((g,))
                    g += 1

            # Software-pipelined emission with 2-group S lookahead: the PE
            # queue is strict in-order, so exp-dependent PV matmuls must
            # not sit ahead of independent S matmuls.  Strict pair/single
            # alternation makes group m's S depend exactly on exp(m-2).
            for idx, gs in enumerate(groups):
                if idx == 0:
                    for gg in groups[0] + groups[1]:
                        s_matmul(gg)
                if len(gs) == 2:
                    pt = pt_pool.tile([KC, 2 * NBQ], BF, tag="ptp")
                    on_dve = K_DVEPAIRS and (idx // 2) % K_DVEPAIRS == 0
                    if on_dve:
                        nc.vector._custom_dve(EXP4, out=pt, in0=ps_A,
                                              in1=b4t, s0=EB1, s1=EB2,
                                              imm2=EB3)
                    else:
                        nc.scalar.activation(pt, ps_A, AF.Exp)
                    pts = [pt[:, 0:NBQ], pt[:, NBQ : 2 * NBQ]]
                else:
                    t = gs[0] % 3
                    stile, sbase = slots[t]
                    src = stile[:, sbase : sbase + NBQ]
                    pt = pt_pool.tile([KC, NBQ], BF, tag="pts")
                    if K_DVEEXP and t == 2:
                        nc.vector._custom_dve(EXP4, out=pt, in0=src,
                                              in1=b4t, s0=EB1, s1=EB2,
                                              imm2=EB3)
                    else:
                        nc.scalar.activation(pt, src, AF.Exp)
                    pts = [pt]
                if idx + 2 < len(groups):
                    for gg in groups[idx + 2]:
                        s_matmul(gg)
                for gg, pt_ap in zip(gs, pts):
                    pv_matmul(gg, pt_ap)
            epilogue(NB - 1, *po[NB - 1])


_CACHE = {}


def _get_program():
    if "nc" not in _CACHE:
        _CACHE["nc"] = build_program()
    return _CACHE["nc"]


def make_in_maps(x, w_qkv):
    """Shard full inputs into per-core input maps. Core i = (b=i//4, h=i%4)."""
    x = np.ascontiguousarray(np.asarray(x, dtype=np.float32))
    w_qkv = np.ascontiguousarray(np.asarray(w_qkv, dtype=np.float32))
    b_, c, d, hh, ww = x.shape
    xf = x.reshape(b_, c, d * hh * ww)
    in_maps = []
    for core in range(NCORES):
        b, h = divmod(core, HEADS)
        rows = np.concatenate([
            np.arange(h * HD, (h + 1) * HD),
            np.arange(C + h * HD, C + (h + 1) * HD),
            np.arange(2 * C + h * HD, 2 * C + (h + 1) * HD),
        ])
        w_h = np.ascontiguousarray(w_qkv[rows, :])          # [48, 64]
        wT_h = np.ascontiguousarray(w_h.T)                   # [64, 48]
        x_b = np.ascontiguousarray(xf[b])                    # [64, 4096]
        x_res = np.ascontiguousarray(x_b[h * HD : (h + 1) * HD])  # [16, 4096]
        # col 0 sums q squares -> partition 0; col 32 sums k squares ->
        # partition 32 (PSUM reads must start 32-aligned)
        ones_pat = np.zeros((2 * HD, 33), dtype=np.float32)
        ones_pat[0:HD, 0] = 1.0
        ones_pat[HD : 2 * HD, 32] = 1.0
        in_maps.append({"x": x_b, "w": w_h, "wT": wT_h, "xres": x_res,
                        "onespat": ones_pat})
    return in_maps


def assemble_output(results, x_shape):
    b_, c, d, hh, ww = x_shape
    out = np.empty((b_, c, d * hh * ww), dtype=np.float32)
    for core in range(NCORES):
        b, h = divmod(core, HEADS)
        out[b, h * HD : (h + 1) * HD] = results[core]["out"]
    return out.reshape(x_shape)


def run(x, w_qkv, trace=False, **kw):
    nc = _get_program()
    in_maps = make_in_maps(x, w_qkv)
    res = run_bass_kernel_spmd(nc, in_maps, list(range(NCORES)),
                               trace=trace, **kw)
    return assemble_output(res.results, np.asarray(x).shape), res


def kernel(x, w_qkv):
    out, _ = run(x, w_qkv)
    return out


# revision 67
# speedup vs baseline: 1.0358x; 1.0358x over previous
"""Trainium2 Bass kernel for nn_Attention_40785009443452.

Reference computation (per batch b):
    qkv = w_qkv @ x_b            # 1x1x1 conv == channel linear
    q,k,v split into 4 heads of dim 16, tokens N = 16*16*16 = 4096
    q,k L2-normalized along head dim
    attn = softmax(q @ k^T)      # [N, N] per (b, head)
    out  = attn @ v  (+ x residual)

Sharding: 8 (batch, head) pairs -> 8 NeuronCores (data + head parallel).
Each core computes one full 4096x4096 attention.

Device algorithm (per core), S^T orientation so softmax reduction (over
keys) lands on the PSUM partition axis and is folded into the PV matmul
via an appended ones-column on V:

    B  = Wq^T Wk                     [64, 64]   (tiny matmul on device)
    G' = (B^T X) * rq  (col scale)   rq[n] = 1/||q_n||
    X' = X * rk                      rk[m] = 1/||k_m||
    S^T chunk [128 keys, 1024 qry] = X'^T(keys) @ G'(cols)  2x concurrent
                                     K=64 matmuls on PE row groups 0/64
    P^T = exp(S^T)                   2 of 3 chunks on ACT; every 3rd on
                                     the DVE via a custom quartic-poly op
                                     (EXP4_ANT; softmax is scale-invariant
                                     and tolerates its ~1.6e-3 rel err)
    O'a [33, 512]  += V'_j^T P^T     queries 0-511,   PE col group 0
    O'b [33, 512]  += V'_j^T P^T     queries 512-1023, PE col group 64
                                     (concurrent col-split PV matmuls)
    out^T = O'[0:16] / O'[32] + x_res

Main-loop PSUM: three S buffers (2 banks each, SEPARATE tiles = separate
conflict domains) + two PV accumulators = exactly 8 banks.  Emission is
software-pipelined with a 2-group S lookahead: the PE queue is strict
in-order, so exp-dependent PV matmuls must never sit ahead of the
independent next S matmuls (head-of-line blocking stalls ACT and lets
HAM re-throttle the PE).  Concurrent same-row-group matmul pairs must
drain to DIFFERENT PSUM banks (same-bank concurrent PE drains are a
fatal collision).

Normalization scales via exp(-0.5*ln(sumsq)) on ACT (Rsqrt/Reciprocal
activations banned for accuracy; Ln+Exp live in one ACT table set).

Measured: 176-180us HW exec (baseline 215us); rel err 5.8e-5.
"""

import os

import numpy as np

import concourse.bass as bass
import concourse.mybir as mybir
import concourse.tile as tile
from concourse import bacc
from concourse.bass_utils import run_bass_kernel_spmd

NCORES = 8
C = 64          # channels
HEADS = 4
HD = 16         # head dim
N = 4096        # tokens (16*16*16)
NBQ = 1024      # queries per block
NB = N // NBQ   # 4 blocks
KC = 128        # keys per chunk
JT = N // KC    # 32 key chunks
FP = mybir.dt.float32
BF = mybir.dt.bfloat16

AF = mybir.ActivationFunctionType

# env kill-switches for risky features
K_PAIR = os.environ.get("K_PAIR", "0") == "1"       # paired-buffer exp
K_PVSPLIT = os.environ.get("K_PVSPLIT", "1") == "1"  # col-group PV split
K_GPMUL = os.environ.get("K_GPMUL", "1") == "1"      # X*rk mul on GPSIMD
K_VREC = os.environ.get("K_VREC", "0") == "1"        # DVE reciprocal epilogue
K_DVEEXP = os.environ.get("K_DVEEXP", "1") == "1"    # singles' exp on DVE
# route every Nth pair's exp to DVE too (0 = none)
K_DVEPAIRS = int(os.environ.get("K_DVEPAIRS", "0"))

# Quartic exp for the DVE (softmax tolerates the ~1.6e-3 rel err):
#   p(x) = 1 + x*(B1 + x*(B2 + x*(B3 + B4*x)))  ~=  exp(x) on [-1.06, 1.06]
# fits the DVE's 8 ALU stages exactly (4 mult + 4 add); B4 rides the
# spilled-C3 slot (a [P,1] tensor read once at element 0).
EB1, EB2, EB3, EB4 = 0.99985291, 0.50492711, 0.17330073, 0.03599347


def _register_exp4():
    from concourse import dve_ops as _dve_ops
    from concourse.dve_spec import (
        Spec, Src0, C0, C1, C2, C3, One, lower, _spill_c3_to_src1, _has_src1,
    )
    from concourse.dve_uop import DveOpSpec

    for op in _dve_ops.OPS:
        if op.name == "EXP4_ANT":
            return op
    x = Src0
    body = _spill_c3_to_src1(One + x * (C0 + x * (C1 + x * (C2 + x * C3))))
    spec = Spec(
        body=body,
        reference=lambda in0, in1, s0, s1, imm2: 1
        + in0 * (s0 + in0 * (s1 + in0 * (imm2 + in0 * in1))),
    )
    shas = {}
    for ver in ("v3", "v4"):
        t = DveOpSpec(name="EXP4_ANT", opcode=0, uops=lower(spec, ver=ver),
                      rd1_en=_has_src1(spec))
        shas[ver] = t.sha(ver)
    op = _dve_ops.DveOp("EXP4_ANT", spec, subdim=False, uops_sha=shas)
    _register_op(op)
    return op


def _register_op(op):
    from concourse import dve_ops as _dve_ops

    _dve_ops.OPS.append(op)
    # the registry dicts are comprehensions over OPS at module import time
    _dve_ops.CUSTOM_DVE_SPECS[op.name] = op.spec
    row = _dve_ops._CUSTOM_DVE_ROW_BASE + len(_dve_ops.OPS) - 1
    assert row < 0x20, "custom DVE row field overflow"
    _dve_ops._SUB_OPCODE_FOR_NAME[op.name] = row


def _register_sq():
    """x^2 with a SINGLE tensor stream, so the input may live in PSUM
    (stock tensor_tensor(x, x) needs two read ports = SBUF only)."""
    from concourse import dve_ops as _dve_ops
    from concourse.dve_spec import Spec, Src0, lower, sq, _has_src1
    from concourse.dve_uop import DveOpSpec

    for op in _dve_ops.OPS:
        if op.name == "SQ1_ANT":
            return op
    spec = Spec(body=sq(Src0), reference=lambda in0: in0 * in0)
    shas = {}
    for ver in ("v3", "v4"):
        t = DveOpSpec(name="SQ1_ANT", opcode=0, uops=lower(spec, ver=ver),
                      rd1_en=_has_src1(spec))
        shas[ver] = t.sha(ver)
    op = _dve_ops.DveOp("SQ1_ANT", spec, subdim=False, uops_sha=shas)
    _register_op(op)
    return op


EXP4 = _register_exp4()
SQ1 = _register_sq()


def build_program():
    nc = bacc.Bacc(
        "TRN2", target_bir_lowering=False, debug=False, enable_asserts=False
    )
    x_d = nc.dram_tensor("x", [C, N], FP, kind="ExternalInput").ap()
    w_d = nc.dram_tensor("w", [3 * HD, C], FP, kind="ExternalInput").ap()
    wT_d = nc.dram_tensor("wT", [C, 3 * HD], FP, kind="ExternalInput").ap()
    xr_d = nc.dram_tensor("xres", [HD, N], FP, kind="ExternalInput").ap()
    op_d = nc.dram_tensor("onespat", [2 * HD, 33], FP,
                          kind="ExternalInput").ap()
    out_d = nc.dram_tensor("out", [HD, N], FP, kind="ExternalOutput").ap()

    with tile.TileContext(nc) as tc:
        _body(tc, x_d, w_d, wT_d, xr_d, op_d, out_d)
    nc.compile()
    return nc


def _body(tc, x_d, w_d, wT_d, xr_d, op_d, out_d):
    nc = tc.nc
    import contextlib

    # Pre-load the ACT table set containing Exp, Ln AND Square so the
    # compiler's per-function chooser doesn't flip-flop between sets.
    if os.environ.get("K_PRELOAD", "1") == "1":
        from concourse.hw_specs import get_activation_tables

        set_names = list(get_activation_tables(nc.m.arch).keys())
        set_id = set_names.index("natural_log_exp_and_others")
        nc.scalar.add_instruction(
            mybir.InstLoadActFuncSet(
                name=f"I-{nc.next_id()}", act_func_set_id=set_id
            )
        )

    with contextlib.ExitStack() as ctx:
        consts = ctx.enter_context(tc.tile_pool(name="consts", bufs=1))

        # ---- load inputs -------------------------------------------------
        # weights on the gpsimd DMA queue; X split across sync+gpsimd
        wq_eng = nc.gpsimd
        WT = consts.tile([C, 3 * HD], FP)
        wq_eng.dma_start(WT, wT_d)
        Wq = consts.tile([HD, C], FP)
        wq_eng.dma_start(Wq, w_d[0:HD, :])
        Wk = consts.tile([HD, C], FP)
        wq_eng.dma_start(Wk, w_d[HD : 2 * HD, :])

        ones1_16 = consts.tile([1, HD], BF)
        nc.any.memset(ones1_16, 1.0)
        ones1_16f = consts.tile([1, HD], FP)
        nc.any.memset(ones1_16f, 1.0)
        ones33 = consts.tile([33, 2 * C], BF)
        nc.any.memset(ones33, 1.0)
        eps33 = consts.tile([33, 1], FP)
        nc.any.memset(eps33, 1e-24)
        b4t = consts.tile([KC, 1], FP)
        nc.any.memset(b4t, EB4)


        # [wqk | wqk] duplicated along BOTH rows (PE row-group alternation
        # across prologue chunks) and cols (qk-proj lands on partitions
        # 0-31 or 32-63 so the sumsq matmul can alternate row strips too)
        WTqk4 = consts.tile([2 * C, 4 * HD], BF)
        for rh in range(2):
            for ch in range(2):
                nc.vector.tensor_copy(
                    WTqk4[rh * C : rh * C + C,
                          ch * 2 * HD : ch * 2 * HD + 2 * HD],
                    WT[:, 0 : 2 * HD])
        ones_pat_f2 = consts.tile([2 * HD, 33], FP)
        nc.sync.dma_start(ones_pat_f2, op_d)
        ones_pat2 = consts.tile([4 * HD, 33], BF)
        nc.vector.tensor_copy(ones_pat2[0 : 2 * HD, :], ones_pat_f2)
        nc.vector.tensor_copy(ones_pat2[2 * HD : 4 * HD, :], ones_pat_f2)
        WTv2 = consts.tile([2 * C, HD], BF)
        nc.vector.tensor_copy(WTv2[0:C, :], WT[:, 2 * HD : 3 * HD])
        nc.vector.tensor_copy(WTv2[C : 2 * C, :], WT[:, 2 * HD : 3 * HD])
        X = consts.tile([C, N], FP)
        Xb2 = consts.tile([2 * C, N], BF)      # bf16 X duplicated rows
        for c8 in range(8):
            sl = slice(c8 * 512, c8 * 512 + 512)
            nc.sync.dma_start(X[:, sl], x_d[:, sl])
            nc.vector.tensor_copy(Xb2[0:C, sl], X[:, sl])
            nc.vector.tensor_copy(Xb2[C : 2 * C, sl], X[:, sl])
        XR = consts.tile([HD, N], FP)
        wq_eng.dma_start(XR, xr_d)

        Bsb2b = consts.tile([2 * C, 2 * C], BF)  # [B | B] dup rows, bf16
        Gp2 = consts.tile([2 * C, N], BF)      # (B^T X)*rq duplicated rows
        Xp2 = consts.tile([2 * C, N], BF)      # X*rk duplicated rows
        # [V_j(16) | zeros | ones@32] stationary tiles for the PV matmul;
        # ones column lands the softmax denominator on PSUM partition 32/96.
        Vp = consts.tile([KC, JT, 33], BF)
        nc.any.memset(Vp, 0.0)
        nc.any.memset(Vp[:, :, 32], 1.0)

        # ---- prologue: B, norms, G', X', V' tiles ------------------------
        # 512-wide chunks through a 6-deep 1-bank slot rotation: a chunk's
        # norm chain never waits on the previous chunk's tail, so the 8
        # chunks pipeline at the per-engine rate (ACT: Ln+Exp, DVE: sq +
        # rep cast + mulG, GPSIMD: X*rk mul, ACT: repk cast).
        with contextlib.ExitStack() as mctx:
            pps = mctx.enter_context(
                tc.tile_pool(name="prol_ps", bufs=6, space="PSUM"))
            psb = mctx.enter_context(tc.tile_pool(name="prol_sb", bufs=2))

            # B = Wq^T Wk (fp32), duplicated into [B | B] on both row halves
            ps_b = pps.tile([C, C], FP, tag="pp2", bufs=3)
            nc.tensor.matmul(ps_b, Wq, Wk, start=True, stop=True)
            for rh in range(2):
                for ch in range(2):
                    nc.vector.tensor_copy(
                        Bsb2b[rh * C : rh * C + C, ch * C : ch * C + C],
                        ps_b)

            for c4 in range(4):
                sl = slice(c4 * 1024, c4 * 1024 + 1024)
                ps_q = pps.tile([2 * HD, 1024], FP, tag="pp2", bufs=3)
                for h2 in range(2):
                    hsl = slice(h2 * 512, h2 * 512 + 512)
                    xsl = slice(c4 * 1024 + h2 * 512,
                                c4 * 1024 + h2 * 512 + 512)
                    rg = slice(h2 * C, h2 * C + C)
                    nc.tensor.matmul(ps_q[:, hsl], WTqk4[rg, 0 : 2 * HD],
                                     Xb2[rg, xsl], start=True, stop=True)
                sqq = psb.tile([2 * HD, 1024], BF, tag="sq")
                nc.scalar.activation(sqq, ps_q, AF.Square)
                ps_nq = pps.tile([33, 1024], FP, tag="pp2", bufs=3)
                for h2 in range(2):
                    hsl = slice(h2 * 512, h2 * 512 + 512)
                    nc.tensor.matmul(ps_nq[:, hsl], ones_pat2[0 : 2 * HD, :],
                                     sqq[:, hsl], start=True, stop=True)
                lnq = psb.tile([33, 1024], FP, tag="ln")
                nc.scalar.activation(lnq, ps_nq, AF.Ln, bias=eps33)
                rqk = psb.tile([33, 1024], BF, tag="rqk")
                nc.scalar.activation(rqk, lnq, AF.Exp, scale=-0.5)
                ps_rep = pps.tile([2 * C, 1024], FP, tag="pp2", bufs=3)
                ps_repk = pps.tile([2 * C, 1024], FP, tag="pp2", bufs=3)
                for h2 in range(2):
                    hsl = slice(h2 * 512, h2 * 512 + 512)
                    nc.tensor.matmul(ps_rep[:, hsl], ones33[0:1, :],
                                     rqk[0:1, hsl], start=True, stop=True)
                    nc.tensor.matmul(ps_repk[:, hsl], ones33[32:33, :],
                                     rqk[32:33, hsl], start=True, stop=True)
                rep_sb = psb.tile([2 * C, 1024], BF, tag="rep")
                nc.vector.tensor_copy(rep_sb, ps_rep)
                repk_sb = psb.tile([2 * C, 1024], BF, tag="repk")
                nc.vector.tensor_copy(repk_sb, ps_repk)
                ps_g = pps.tile([2 * C, 1024], FP, tag="pp2", bufs=3)
                for h2 in range(2):
                    hsl = slice(h2 * 512, h2 * 512 + 512)
                    xsl = slice(c4 * 1024 + h2 * 512,
                                c4 * 1024 + h2 * 512 + 512)
                    rg = slice(h2 * C, h2 * C + C)
                    nc.tensor.matmul(ps_g[:, hsl], Bsb2b[rg, :],
                                     Xb2[rg, xsl], start=True, stop=True)
                nc.vector.tensor_mul(Gp2[:, sl], ps_g, rep_sb)
                if K_GPMUL:
                    nc.gpsimd.tensor_mul(Xp2[:, sl], Xb2[:, sl], repk_sb)
                else:
                    nc.vector.tensor_mul(Xp2[0:C, sl], ps_repk[0:C, :],
                                         X[:, sl])
                    nc.vector.tensor_mul(Xp2[C : 2 * C, sl],
                                         ps_repk[C : 2 * C, :], X[:, sl])

                ps_kv8 = pps.tile([KC, 8, HD], FP, tag="ppv", bufs=2)
                for jj in range(8):
                    j = 8 * c4 + jj
                    ksl = slice(j * KC, j * KC + KC)
                    nc.tensor.matmul(ps_kv8[:, jj, :], Xb2[0:C, ksl],
                                     WTv2[0:C, :], start=True, stop=True)
                nc.vector.tensor_copy(
                    Vp[:, 8 * c4 : 8 * c4 + 8, 0:HD], ps_kv8)
        # ---- main attention loop ----------------------------------------
        with contextlib.ExitStack() as mctx:
            ps_pool = mctx.enter_context(
                tc.tile_pool(name="ps_main", bufs=1, space="PSUM"))
            pt_pool = mctx.enter_context(tc.tile_pool(name="pt", bufs=12))
            ep_pool = mctx.enter_context(tc.tile_pool(name="ep", bufs=2))

            # S^T triple buffer: pair slots as ONE flat [128, 2048] when
            # pairing (the pair-exp AP must be contiguous in one tile),
            # else three separate tiles (= separate PSUM conflict domains).
            if K_PAIR:
                ps_A = ps_pool.tile([KC, 2 * NBQ], FP, tag="ps_A")
                ps_B = ps_pool.tile([KC, NBQ], FP, tag="ps_B")
                slots = [(ps_A, 0), (ps_A, NBQ), (ps_B, 0)]
            else:
                ps_A0 = ps_pool.tile([KC, NBQ], FP, tag="ps_A0")
                ps_A1 = ps_pool.tile([KC, NBQ], FP, tag="ps_A1")
                ps_B = ps_pool.tile([KC, NBQ], FP, tag="ps_B")
                ps_A = None
                slots = [(ps_A0, 0), (ps_A1, 0), (ps_B, 0)]

            def epilogue(nb, po_a, po_b):
                nbase = nb * NBQ
                oall = ep_pool.tile([33, NBQ], FP, tag="oall",
                                    name=f"oall_{nb}")
                # Ln reads the denominator rows straight from PSUM (32- and
                # 96-aligned), in parallel with the oall evacuation
                lnd2 = ep_pool.tile([1, NBQ], FP, tag="lnd2",
                                    name=f"lnd2_{nb}")
                nc.scalar.activation(lnd2[:, 0:512], po_a[32:33, :], AF.Ln)
                nc.scalar.activation(lnd2[:, 512:1024], po_b[96:97, :],
                                     AF.Ln)
                nc.vector.tensor_copy(oall[:, 0:512], po_a)
                nc.vector.tensor_copy(oall[:, 512:1024], po_b[C:97, :])
                if K_VREC:
                    rinv = ep_pool.tile([1, NBQ], FP, tag="rinv",
                                        name=f"rinv_{nb}")
                    nc.vector.reciprocal(rinv, oall[32:33, :])
                    rep_lhs = ones1_16f
                else:
                    rinv = ep_pool.tile([1, NBQ], BF, tag="rinv",
                                        name=f"rinv_{nb}")
                    nc.scalar.activation(rinv, lnd2, AF.Exp, scale=-1.0)
                    rep_lhs = ones1_16
                t2 = ep_pool.tile([HD, NBQ], FP, tag="t2", name=f"t2_{nb}")
                for h2 in range(2):
                    qsl = slice(h2 * 512, h2 * 512 + 512)
                    tag = "po_a" if h2 == 0 else "po_b"
                    shape = [HD, 512] if h2 == 0 else [C + HD, 512]
                    ps_rep = ps_pool.tile(shape, FP, tag=tag,
                                          name=f"ps_rep_{nb}_{h2}")
                    pr = ps_rep if h2 == 0 else ps_rep[C : C + HD, :]
                    nc.tensor.matmul(pr, rep_lhs, rinv[:, qsl],
                                     start=True, stop=True)
                    nc.vector.tensor_mul(t2[:, qsl], oall[0:HD, qsl], pr)
                osb = ep_pool.tile([HD, NBQ], FP, tag="osb",
                                   name=f"osb_{nb}")
                osl = slice(nbase, nbase + NBQ)
                nc.vector.tensor_add(osb, t2, XR[:, osl])
                nc.sync.dma_start(out_d[:, osl], osb)

            def s_matmul(g):
                nb, j = divmod(g, JT)
                nbase = nb * NBQ
                t = g % 3
                ksl = slice(j * KC, j * KC + KC)
                stile, sbase = slots[t]
                for h2 in range(2):
                    gsl = slice(nbase + h2 * 512, nbase + h2 * 512 + 512)
                    rg = slice(h2 * C, h2 * C + C)  # alternate row groups
                    out = stile[:, sbase + h2 * 512 : sbase + h2 * 512 + 512]
                    nc.tensor.matmul(out, Xp2[rg, ksl],
                                     Gp2[rg, gsl], start=True, stop=True)

            po = {}

            def ensure_po(nb):
                if nb not in po:
                    po_a = ps_pool.tile([33, 512], FP, tag="po_a",
                                        name=f"po_a_{nb}")
                    po_b = ps_pool.tile([97, 512], FP, tag="po_b",
                                        name=f"po_b_{nb}")
                    po[nb] = (po_a, po_b)
                return po[nb]

            def pv_matmul(g, pt_ap):
                nb, j = divmod(g, JT)
                # PV accumulators are single-buffered: emit the previous
                # block's epilogue (its readers) before this block's first
                # PV write, even when an exp pair straddles two blocks.
                if j == 0 and nb > 0:
                    epilogue(nb - 1, *po[nb - 1])
                po_a, po_b = ensure_po(nb)
                first, last = j == 0, j == JT - 1
                nc.tensor.matmul(po_a, Vp[:, j, :], pt_ap[:, 0:512],
                                 start=first, stop=last)
                if K_PVSPLIT:
                    nc.tensor.matmul(po_b[C:97, :], Vp[:, j, :],
                                     pt_ap[:, 512:1024],
                                     start=first, stop=last)
                else:
                    nc.tensor.matmul(po_b[0:33, :], Vp[:, j, :],
                                     pt_ap[:, 512:1024],
                                     start=first, stop=last)

            # GLOBAL chunk index g over all blocks so the pair/single
            # pattern alternates strictly: P(0,1) S(2) P(0,1) S(2) ...
            # Slots (0,1) are consumed by paired exps (FD=2048), slot 2
            # by singles.  A pair may straddle two query blocks.
            CH = NB * JT
            groups = []
            g = 0
            while g < CH:
                if K_PAIR and g % 3 < 2 and g + 1 < CH:
                    groups.append((g, g + 1))
                    g += 2
                else:
                    groups.append((g,))
                    g += 1

            # Software-pipelined emission with 2-group S lookahead: the PE
            # queue is strict in-order, so exp-dependent PV matmuls must
            # not sit ahead of independent S matmuls.  Strict pair/single
            # alternation makes group m's S depend exactly on exp(m-2).
            for idx, gs in enumerate(groups):
                if idx == 0:
                    for gg in groups[0] + groups[1] + groups[2]:
                        s_matmul(gg)
                if len(gs) == 2:
                    pt = pt_pool.tile([KC, 2 * NBQ], BF, tag="ptp")
                    on_dve = K_DVEPAIRS and (idx // 2) % K_DVEPAIRS == 0
                    if on_dve:
                        nc.vector._custom_dve(EXP4, out=pt, in0=ps_A,
                                              in1=b4t, s0=EB1, s1=EB2,
                                              imm2=EB3)
                    else:
                        nc.scalar.activation(pt, ps_A, AF.Exp)
                    pts = [pt[:, 0:NBQ], pt[:, NBQ : 2 * NBQ]]
                else:
                    t = gs[0] % 3
                    stile, sbase = slots[t]
                    src = stile[:, sbase : sbase + NBQ]
                    pt = pt_pool.tile([KC, NBQ], BF, tag="pts")
                    if K_DVEEXP and t == 2:
                        nc.vector._custom_dve(EXP4, out=pt, in0=src,
                                              in1=b4t, s0=EB1, s1=EB2,
                                              imm2=EB3)
                    else:
                        nc.scalar.activation(pt, src, AF.Exp)
                    pts = [pt]
                if idx + 3 < len(groups):
                    for gg in groups[idx + 3]:
                        s_matmul(gg)
                for gg, pt_ap in zip(gs, pts):
                    pv_matmul(gg, pt_ap)
            epilogue(NB - 1, *po[NB - 1])


_CACHE = {}


def _get_program():
    if "nc" not in _CACHE:
        _CACHE["nc"] = build_program()
    return _CACHE["nc"]


def make_in_maps(x, w_qkv):
    """Shard full inputs into per-core input maps. Core i = (b=i//4, h=i%4)."""
    x = np.ascontiguousarray(np.asarray(x, dtype=np.float32))
    w_qkv = np.ascontiguousarray(np.asarray(w_qkv, dtype=np.float32))
    b_, c, d, hh, ww = x.shape
    xf = x.reshape(b_, c, d * hh * ww)
    in_maps = []
    for core in range(NCORES):
        b, h = divmod(core, HEADS)
        rows = np.concatenate([
            np.arange(h * HD, (h + 1) * HD),
            np.arange(C + h * HD, C + (h + 1) * HD),
            np.arange(2 * C + h * HD, 2 * C + (h + 1) * HD),
        ])
        w_h = np.ascontiguousarray(w_qkv[rows, :])          # [48, 64]
        wT_h = np.ascontiguousarray(w_h.T)                   # [64, 48]
        x_b = np.ascontiguousarray(xf[b])                    # [64, 4096]
        x_res = np.ascontiguousarray(x_b[h * HD : (h + 1) * HD])  # [16, 4096]
        # col 0 sums q squares -> partition 0; col 32 sums k squares ->
        # partition 32 (PSUM reads must start 32-aligned)
        ones_pat = np.zeros((2 * HD, 33), dtype=np.float32)
        ones_pat[0:HD, 0] = 1.0
        ones_pat[HD : 2 * HD, 32] = 1.0
        in_maps.append({"x": x_b, "w": w_h, "wT": wT_h, "xres": x_res,
                        "onespat": ones_pat})
    return in_maps


def assemble_output(results, x_shape):
    b_, c, d, hh, ww = x_shape
    out = np.empty((b_, c, d * hh * ww), dtype=np.float32)
    for core in range(NCORES):
        b, h = divmod(core, HEADS)
        out[b, h * HD : (h + 1) * HD] = results[core]["out"]
    return out.reshape(x_shape)


def run(x, w_qkv, trace=False, **kw):
    nc = _get_program()
    in_maps = make_in_maps(x, w_qkv)
    res = run_bass_kernel_spmd(nc, in_maps, list(range(NCORES)),
                               trace=trace, **kw)
    return assemble_output(res.results, np.asarray(x).shape), res


def kernel(x, w_qkv):
    out, _ = run(x, w_qkv)
    return out


# revision 68
# speedup vs baseline: 1.0461x; 1.0099x over previous
"""Trainium2 Bass kernel for nn_Attention_40785009443452.

Reference computation (per batch b):
    qkv = w_qkv @ x_b            # 1x1x1 conv == channel linear
    q,k,v split into 4 heads of dim 16, tokens N = 16*16*16 = 4096
    q,k L2-normalized along head dim
    attn = softmax(q @ k^T)      # [N, N] per (b, head)
    out  = attn @ v  (+ x residual)

Sharding: 8 (batch, head) pairs -> 8 NeuronCores (data + head parallel).
Each core computes one full 4096x4096 attention.

Device algorithm (per core), S^T orientation so softmax reduction (over
keys) lands on the PSUM partition axis and is folded into the PV matmul
via an appended ones-column on V:

    B  = Wq^T Wk                     [64, 64]   (tiny matmul on device)
    G' = (B^T X) * rq  (col scale)   rq[n] = 1/||q_n||
    X' = X * rk                      rk[m] = 1/||k_m||
    S^T chunk [128 keys, 1024 qry] = X'^T(keys) @ G'(cols)  2x concurrent
                                     K=64 matmuls on PE row groups 0/64
    P^T = exp(S^T)                   2 of 3 chunks on ACT; every 3rd on
                                     the DVE via a custom quartic-poly op
                                     (EXP4_ANT; softmax is scale-invariant
                                     and tolerates its ~1.6e-3 rel err)
    O'a [33, 512]  += V'_j^T P^T     queries 0-511,   PE col group 0
    O'b [33, 512]  += V'_j^T P^T     queries 512-1023, PE col group 64
                                     (concurrent col-split PV matmuls)
    out^T = O'[0:16] / O'[32] + x_res

Main-loop PSUM: three S buffers (2 banks each, SEPARATE tiles = separate
conflict domains) + two PV accumulators = exactly 8 banks.  Emission is
software-pipelined with a 2-group S lookahead: the PE queue is strict
in-order, so exp-dependent PV matmuls must never sit ahead of the
independent next S matmuls (head-of-line blocking stalls ACT and lets
HAM re-throttle the PE).  Concurrent same-row-group matmul pairs must
drain to DIFFERENT PSUM banks (same-bank concurrent PE drains are a
fatal collision).

Normalization scales via exp(-0.5*ln(sumsq)) on ACT (Rsqrt/Reciprocal
activations banned for accuracy; Ln+Exp live in one ACT table set).

Measured: ~164-170us HW exec (baseline 215us); rel err 5.8e-5.
"""

import os

import numpy as np

import concourse.bass as bass
import concourse.mybir as mybir
import concourse.tile as tile
from concourse import bacc
from concourse.bass_utils import run_bass_kernel_spmd

NCORES = 8
C = 64          # channels
HEADS = 4
HD = 16         # head dim
N = 4096        # tokens (16*16*16)
NBQ = 1024      # queries per block
NB = N // NBQ   # 4 blocks
KC = 128        # keys per chunk
JT = N // KC    # 32 key chunks
FP = mybir.dt.float32
BF = mybir.dt.bfloat16

AF = mybir.ActivationFunctionType

# env kill-switches for risky features
K_PAIR = os.environ.get("K_PAIR", "0") == "1"       # paired-buffer exp
K_PVSPLIT = os.environ.get("K_PVSPLIT", "1") == "1"  # col-group PV split
K_GPMUL = os.environ.get("K_GPMUL", "1") == "1"      # X*rk mul on GPSIMD
K_VREC = os.environ.get("K_VREC", "0") == "1"        # DVE reciprocal epilogue
K_DVEEXP = os.environ.get("K_DVEEXP", "1") == "1"    # singles' exp on DVE
# route every Nth pair's exp to DVE too (0 = none)
K_DVEPAIRS = int(os.environ.get("K_DVEPAIRS", "0"))

# Quartic exp for the DVE (softmax tolerates the ~1.6e-3 rel err):
#   p(x) = 1 + x*(B1 + x*(B2 + x*(B3 + B4*x)))  ~=  exp(x) on [-1.06, 1.06]
# fits the DVE's 8 ALU stages exactly (4 mult + 4 add); B4 rides the
# spilled-C3 slot (a [P,1] tensor read once at element 0).
EB1, EB2, EB3, EB4 = 0.99985291, 0.50492711, 0.17330073, 0.03599347


def _register_exp4():
    from concourse import dve_ops as _dve_ops
    from concourse.dve_spec import (
        Spec, Src0, C0, C1, C2, C3, One, lower, _spill_c3_to_src1, _has_src1,
    )
    from concourse.dve_uop import DveOpSpec

    for op in _dve_ops.OPS:
        if op.name == "EXP4_ANT":
            return op
    x = Src0
    body = _spill_c3_to_src1(One + x * (C0 + x * (C1 + x * (C2 + x * C3))))
    spec = Spec(
        body=body,
        reference=lambda in0, in1, s0, s1, imm2: 1
        + in0 * (s0 + in0 * (s1 + in0 * (imm2 + in0 * in1))),
    )
    shas = {}
    for ver in ("v3", "v4"):
        t = DveOpSpec(name="EXP4_ANT", opcode=0, uops=lower(spec, ver=ver),
                      rd1_en=_has_src1(spec))
        shas[ver] = t.sha(ver)
    op = _dve_ops.DveOp("EXP4_ANT", spec, subdim=False, uops_sha=shas)
    _register_op(op)
    return op


def _register_op(op):
    from concourse import dve_ops as _dve_ops

    _dve_ops.OPS.append(op)
    # the registry dicts are comprehensions over OPS at module import time
    _dve_ops.CUSTOM_DVE_SPECS[op.name] = op.spec
    row = _dve_ops._CUSTOM_DVE_ROW_BASE + len(_dve_ops.OPS) - 1
    assert row < 0x20, "custom DVE row field overflow"
    _dve_ops._SUB_OPCODE_FOR_NAME[op.name] = row


def _register_sq():
    """x^2 with a SINGLE tensor stream, so the input may live in PSUM
    (stock tensor_tensor(x, x) needs two read ports = SBUF only)."""
    from concourse import dve_ops as _dve_ops
    from concourse.dve_spec import Spec, Src0, lower, sq, _has_src1
    from concourse.dve_uop import DveOpSpec

    for op in _dve_ops.OPS:
        if op.name == "SQ1_ANT":
            return op
    spec = Spec(body=sq(Src0), reference=lambda in0: in0 * in0)
    shas = {}
    for ver in ("v3", "v4"):
        t = DveOpSpec(name="SQ1_ANT", opcode=0, uops=lower(spec, ver=ver),
                      rd1_en=_has_src1(spec))
        shas[ver] = t.sha(ver)
    op = _dve_ops.DveOp("SQ1_ANT", spec, subdim=False, uops_sha=shas)
    _register_op(op)
    return op


EXP4 = _register_exp4()
SQ1 = _register_sq()


def build_program():
    nc = bacc.Bacc(
        "TRN2", target_bir_lowering=False, debug=False, enable_asserts=False
    )
    x_d = nc.dram_tensor("x", [C, N], FP, kind="ExternalInput").ap()
    w_d = nc.dram_tensor("w", [3 * HD, C], FP, kind="ExternalInput").ap()
    wT_d = nc.dram_tensor("wT", [C, 3 * HD], FP, kind="ExternalInput").ap()
    xr_d = nc.dram_tensor("xres", [HD, N], FP, kind="ExternalInput").ap()
    op_d = nc.dram_tensor("onespat", [2 * HD, 33], FP,
                          kind="ExternalInput").ap()
    out_d = nc.dram_tensor("out", [HD, N], FP, kind="ExternalOutput").ap()

    with tile.TileContext(nc) as tc:
        _body(tc, x_d, w_d, wT_d, xr_d, op_d, out_d)
    nc.compile()
    return nc


def _body(tc, x_d, w_d, wT_d, xr_d, op_d, out_d):
    nc = tc.nc
    import contextlib

    # Pre-load the ACT table set containing Exp, Ln AND Square so the
    # compiler's per-function chooser doesn't flip-flop between sets.
    if os.environ.get("K_PRELOAD", "1") == "1":
        from concourse.hw_specs import get_activation_tables

        set_names = list(get_activation_tables(nc.m.arch).keys())
        set_id = set_names.index("natural_log_exp_and_others")
        nc.scalar.add_instruction(
            mybir.InstLoadActFuncSet(
                name=f"I-{nc.next_id()}", act_func_set_id=set_id
            )
        )

    with contextlib.ExitStack() as ctx:
        consts = ctx.enter_context(tc.tile_pool(name="consts", bufs=1))

        # ---- load inputs -------------------------------------------------
        # weights on the gpsimd DMA queue; X split across sync+gpsimd
        wq_eng = nc.gpsimd
        WT = consts.tile([C, 3 * HD], FP)
        wq_eng.dma_start(WT, wT_d)
        Wq = consts.tile([HD, C], FP)
        wq_eng.dma_start(Wq, w_d[0:HD, :])
        Wk = consts.tile([HD, C], FP)
        wq_eng.dma_start(Wk, w_d[HD : 2 * HD, :])

        ones1_16 = consts.tile([1, HD], BF)
        nc.any.memset(ones1_16, 1.0)
        ones1_16f = consts.tile([1, HD], FP)
        nc.any.memset(ones1_16f, 1.0)
        ones33 = consts.tile([33, 2 * C], BF)
        nc.any.memset(ones33, 1.0)
        eps33 = consts.tile([33, 1], FP)
        nc.any.memset(eps33, 1e-24)
        b4t = consts.tile([KC, 1], FP)
        nc.any.memset(b4t, EB4)


        # [wqk | wqk] duplicated along BOTH rows (PE row-group alternation
        # across prologue chunks) and cols (qk-proj lands on partitions
        # 0-31 or 32-63 so the sumsq matmul can alternate row strips too)
        WTqk4 = consts.tile([2 * C, 4 * HD], BF)
        for rh in range(2):
            for ch in range(2):
                nc.vector.tensor_copy(
                    WTqk4[rh * C : rh * C + C,
                          ch * 2 * HD : ch * 2 * HD + 2 * HD],
                    WT[:, 0 : 2 * HD])
        ones_pat_f2 = consts.tile([2 * HD, 33], FP)
        nc.sync.dma_start(ones_pat_f2, op_d)
        ones_pat2 = consts.tile([4 * HD, 33], BF)
        nc.vector.tensor_copy(ones_pat2[0 : 2 * HD, :], ones_pat_f2)
        nc.vector.tensor_copy(ones_pat2[2 * HD : 4 * HD, :], ones_pat_f2)
        WTv2 = consts.tile([2 * C, HD], BF)
        nc.vector.tensor_copy(WTv2[0:C, :], WT[:, 2 * HD : 3 * HD])
        nc.vector.tensor_copy(WTv2[C : 2 * C, :], WT[:, 2 * HD : 3 * HD])
        X = consts.tile([C, N], FP)
        Xb2 = consts.tile([2 * C, N], BF)      # bf16 X duplicated rows
        for c8 in range(8):
            sl = slice(c8 * 512, c8 * 512 + 512)
            nc.sync.dma_start(X[:, sl], x_d[:, sl])
            nc.vector.tensor_copy(Xb2[0:C, sl], X[:, sl])
            nc.vector.tensor_copy(Xb2[C : 2 * C, sl], X[:, sl])
        XR = consts.tile([HD, N], FP)
        wq_eng.dma_start(XR, xr_d)

        Bsb2b = consts.tile([2 * C, 2 * C], BF)  # [B | B] dup rows, bf16
        Gp2 = consts.tile([2 * C, N], BF)      # (B^T X)*rq duplicated rows
        Xp2 = consts.tile([2 * C, N], BF)      # X*rk duplicated rows
        # [V_j(16) | zeros | ones@32] stationary tiles for the PV matmul;
        # ones column lands the softmax denominator on PSUM partition 32/96.
        Vp = consts.tile([KC, JT, 33], BF)
        nc.any.memset(Vp, 0.0)
        nc.any.memset(Vp[:, :, 32], 1.0)

        # ---- prologue: B, norms, G', X', V' tiles ------------------------
        # 512-wide chunks through a 6-deep 1-bank slot rotation: a chunk's
        # norm chain never waits on the previous chunk's tail, so the 8
        # chunks pipeline at the per-engine rate (ACT: Ln+Exp, DVE: sq +
        # rep cast + mulG, GPSIMD: X*rk mul, ACT: repk cast).
        with contextlib.ExitStack() as mctx:
            pps = mctx.enter_context(
                tc.tile_pool(name="prol_ps", bufs=6, space="PSUM"))
            psb = mctx.enter_context(tc.tile_pool(name="prol_sb", bufs=2))

            # B = Wq^T Wk (fp32), duplicated into [B | B] on both row halves
            ps_b = pps.tile([C, C], FP, tag="pp2", bufs=3)
            nc.tensor.matmul(ps_b, Wq, Wk, start=True, stop=True)
            for rh in range(2):
                for ch in range(2):
                    nc.vector.tensor_copy(
                        Bsb2b[rh * C : rh * C + C, ch * C : ch * C + C],
                        ps_b)

            for c4 in range(4):
                sl = slice(c4 * 1024, c4 * 1024 + 1024)
                ps_q = pps.tile([2 * HD, 1024], FP, tag="pp2", bufs=3)
                for h2 in range(2):
                    hsl = slice(h2 * 512, h2 * 512 + 512)
                    xsl = slice(c4 * 1024 + h2 * 512,
                                c4 * 1024 + h2 * 512 + 512)
                    rg = slice(h2 * C, h2 * C + C)
                    nc.tensor.matmul(ps_q[:, hsl], WTqk4[rg, 0 : 2 * HD],
                                     Xb2[rg, xsl], start=True, stop=True)
                sqq = psb.tile([2 * HD, 1024], BF, tag="sq")
                nc.scalar.activation(sqq, ps_q, AF.Square)
                ps_nq = pps.tile([33, 1024], FP, tag="pp2", bufs=3)
                for h2 in range(2):
                    hsl = slice(h2 * 512, h2 * 512 + 512)
                    nc.tensor.matmul(ps_nq[:, hsl], ones_pat2[0 : 2 * HD, :],
                                     sqq[:, hsl], start=True, stop=True)
                lnq = psb.tile([33, 1024], FP, tag="ln")
                nc.scalar.activation(lnq, ps_nq, AF.Ln, bias=eps33)
                rqk = psb.tile([33, 1024], BF, tag="rqk")
                nc.scalar.activation(rqk, lnq, AF.Exp, scale=-0.5)
                ps_rep = pps.tile([2 * C, 1024], FP, tag="pp2", bufs=3)
                ps_repk = pps.tile([2 * C, 1024], FP, tag="pp2", bufs=3)
                for h2 in range(2):
                    hsl = slice(h2 * 512, h2 * 512 + 512)
                    nc.tensor.matmul(ps_rep[:, hsl], ones33[0:1, :],
                                     rqk[0:1, hsl], start=True, stop=True)
                    nc.tensor.matmul(ps_repk[:, hsl], ones33[32:33, :],
                                     rqk[32:33, hsl], start=True, stop=True)
                rep_sb = psb.tile([2 * C, 1024], BF, tag="rep")
                nc.vector.tensor_copy(rep_sb, ps_rep)
                repk_sb = psb.tile([2 * C, 1024], BF, tag="repk")
                nc.vector.tensor_copy(repk_sb, ps_repk)
                ps_g = pps.tile([2 * C, 1024], FP, tag="pp2", bufs=3)
                for h2 in range(2):
                    hsl = slice(h2 * 512, h2 * 512 + 512)
                    xsl = slice(c4 * 1024 + h2 * 512,
                                c4 * 1024 + h2 * 512 + 512)
                    rg = slice(h2 * C, h2 * C + C)
                    nc.tensor.matmul(ps_g[:, hsl], Bsb2b[rg, :],
                                     Xb2[rg, xsl], start=True, stop=True)
                nc.vector.tensor_mul(Gp2[:, sl], ps_g, rep_sb)
                if K_GPMUL:
                    nc.gpsimd.tensor_mul(Xp2[:, sl], Xb2[:, sl], repk_sb)
                else:
                    nc.vector.tensor_mul(Xp2[0:C, sl], ps_repk[0:C, :],
                                         X[:, sl])
                    nc.vector.tensor_mul(Xp2[C : 2 * C, sl],
                                         ps_repk[C : 2 * C, :], X[:, sl])

                ps_kv8 = pps.tile([KC, 8, HD], FP, tag="ppv", bufs=2)
                for jj in range(8):
                    j = 8 * c4 + jj
                    ksl = slice(j * KC, j * KC + KC)
                    nc.tensor.matmul(ps_kv8[:, jj, :], Xb2[0:C, ksl],
                                     WTv2[0:C, :], start=True, stop=True)
                nc.vector.tensor_copy(
                    Vp[:, 8 * c4 : 8 * c4 + 8, 0:HD], ps_kv8)
        # ---- main attention loop ----------------------------------------
        with contextlib.ExitStack() as mctx:
            ps_pool = mctx.enter_context(
                tc.tile_pool(name="ps_main", bufs=1, space="PSUM"))
            pt_pool = mctx.enter_context(tc.tile_pool(name="pt", bufs=12))
            ep_pool = mctx.enter_context(tc.tile_pool(name="ep", bufs=2))

            # S^T triple buffer: pair slots as ONE flat [128, 2048] when
            # pairing (the pair-exp AP must be contiguous in one tile),
            # else three separate tiles (= separate PSUM conflict domains).
            if K_PAIR:
                ps_A = ps_pool.tile([KC, 2 * NBQ], FP, tag="ps_A")
                ps_B = ps_pool.tile([KC, NBQ], FP, tag="ps_B")
                slots = [(ps_A, 0), (ps_A, NBQ), (ps_B, 0)]
            else:
                ps_A0 = ps_pool.tile([KC, NBQ], FP, tag="ps_A0")
                ps_A1 = ps_pool.tile([KC, NBQ], FP, tag="ps_A1")
                ps_B = ps_pool.tile([KC, NBQ], FP, tag="ps_B")
                ps_A = None
                slots = [(ps_A0, 0), (ps_A1, 0), (ps_B, 0)]

            def epilogue(nb, po_a, po_b):
                nbase = nb * NBQ
                oall = ep_pool.tile([33, NBQ], FP, tag="oall",
                                    name=f"oall_{nb}")
                # Ln reads the denominator rows straight from PSUM (32- and
                # 96-aligned), in parallel with the oall evacuation
                lnd2 = ep_pool.tile([1, NBQ], FP, tag="lnd2",
                                    name=f"lnd2_{nb}")
                nc.scalar.activation(lnd2[:, 0:512], po_a[32:33, :], AF.Ln)
                nc.scalar.activation(lnd2[:, 512:1024], po_b[96:97, :],
                                     AF.Ln)
                nc.vector.tensor_copy(oall[:, 0:512], po_a)
                nc.vector.tensor_copy(oall[:, 512:1024], po_b[C:97, :])
                if K_VREC:
                    rinv = ep_pool.tile([1, NBQ], FP, tag="rinv",
                                        name=f"rinv_{nb}")
                    nc.vector.reciprocal(rinv, oall[32:33, :])
                    rep_lhs = ones1_16f
                else:
                    rinv = ep_pool.tile([1, NBQ], BF, tag="rinv",
                                        name=f"rinv_{nb}")
                    nc.scalar.activation(rinv, lnd2, AF.Exp, scale=-1.0)
                    rep_lhs = ones1_16
                t2 = ep_pool.tile([HD, NBQ], FP, tag="t2", name=f"t2_{nb}")
                for h2 in range(2):
                    qsl = slice(h2 * 512, h2 * 512 + 512)
                    tag = "po_a" if h2 == 0 else "po_b"
                    shape = [HD, 512] if h2 == 0 else [C + HD, 512]
                    ps_rep = ps_pool.tile(shape, FP, tag=tag,
                                          name=f"ps_rep_{nb}_{h2}")
                    pr = ps_rep if h2 == 0 else ps_rep[C : C + HD, :]
                    nc.tensor.matmul(pr, rep_lhs, rinv[:, qsl],
                                     start=True, stop=True)
                    nc.vector.tensor_mul(t2[:, qsl], oall[0:HD, qsl], pr)
                osb = ep_pool.tile([HD, NBQ], FP, tag="osb",
                                   name=f"osb_{nb}")
                osl = slice(nbase, nbase + NBQ)
                nc.vector.tensor_add(osb, t2, XR[:, osl])
                nc.sync.dma_start(out_d[:, osl], osb)

            def s_matmul(g):
                nb, j = divmod(g, JT)
                nbase = nb * NBQ
                t = g % 3
                ksl = slice(j * KC, j * KC + KC)
                stile, sbase = slots[t]
                for h2 in range(2):
                    gsl = slice(nbase + h2 * 512, nbase + h2 * 512 + 512)
                    rg = slice(h2 * C, h2 * C + C)  # alternate row groups
                    out = stile[:, sbase + h2 * 512 : sbase + h2 * 512 + 512]
                    nc.tensor.matmul(out, Xp2[rg, ksl],
                                     Gp2[rg, gsl], start=True, stop=True)

            po = {}

            def ensure_po(nb):
                if nb not in po:
                    po_a = ps_pool.tile([33, 512], FP, tag="po_a",
                                        name=f"po_a_{nb}")
                    po_b = ps_pool.tile([97, 512], FP, tag="po_b",
                                        name=f"po_b_{nb}")
                    po[nb] = (po_a, po_b)
                return po[nb]

            def pv_matmul(g, pt_ap):
                nb, j = divmod(g, JT)
                # PV accumulators are single-buffered: emit the previous
                # block's epilogue (its readers) before this block's first
                # PV write, even when an exp pair straddles two blocks.
                if j == 0 and nb > 0:
                    epilogue(nb - 1, *po[nb - 1])
                po_a, po_b = ensure_po(nb)
                first, last = j == 0, j == JT - 1
                nc.tensor.matmul(po_a, Vp[:, j, :], pt_ap[:, 0:512],
                                 start=first, stop=last)
                if K_PVSPLIT:
                    nc.tensor.matmul(po_b[C:97, :], Vp[:, j, :],
                                     pt_ap[:, 512:1024],
                                     start=first, stop=last)
                else:
                    nc.tensor.matmul(po_b[0:33, :], Vp[:, j, :],
                                     pt_ap[:, 512:1024],
                                     start=first, stop=last)

            # GLOBAL chunk index g over all blocks so the pair/single
            # pattern alternates strictly: P(0,1) S(2) P(0,1) S(2) ...
            # Slots (0,1) are consumed by paired exps (FD=2048), slot 2
            # by singles.  A pair may straddle two query blocks.
            CH = NB * JT
            groups = []
            g = 0
            while g < CH:
                if K_PAIR and g % 3 < 2 and g + 1 < CH:
                    groups.append((g, g + 1))
                    g += 2
                else:
                    groups.append((g,))
                    g += 1

            # Software-pipelined emission with 3-group S lookahead: the PE
            # queue is strict in-order, so exp-dependent PV matmuls must
            # not sit ahead of independent S matmuls.  3 is the emission-
            # order safety limit: S(g)'s slot is freed by exp(g-3), which
            # is emitted at the top of the same iteration that emits S(g).
            for idx, gs in enumerate(groups):
                if idx == 0:
                    for gg in groups[0] + groups[1] + groups[2]:
                        s_matmul(gg)
                if len(gs) == 2:
                    pt = pt_pool.tile([KC, 2 * NBQ], BF, tag="ptp")
                    on_dve = K_DVEPAIRS and (idx // 2) % K_DVEPAIRS == 0
                    if on_dve:
                        nc.vector._custom_dve(EXP4, out=pt, in0=ps_A,
                                              in1=b4t, s0=EB1, s1=EB2,
                                              imm2=EB3)
                    else:
                        nc.scalar.activation(pt, ps_A, AF.Exp)
                    pts = [pt[:, 0:NBQ], pt[:, NBQ : 2 * NBQ]]
                else:
                    t = gs[0] % 3
                    stile, sbase = slots[t]
                    src = stile[:, sbase : sbase + NBQ]
                    pt = pt_pool.tile([KC, NBQ], BF, tag="pts")
                    if K_DVEEXP and t == 2:
                        nc.vector._custom_dve(EXP4, out=pt, in0=src,
                                              in1=b4t, s0=EB1, s1=EB2,
                                              imm2=EB3)
                    else:
                        nc.scalar.activation(pt, src, AF.Exp)
                    pts = [pt]
                if idx + 3 < len(groups):
                    for gg in groups[idx + 3]:
                        s_matmul(gg)
                for gg, pt_ap in zip(gs, pts):
                    pv_matmul(gg, pt_ap)
            epilogue(NB - 1, *po[NB - 1])


_CACHE = {}


def _get_program():
    if "nc" not in _CACHE:
        _CACHE["nc"] = build_program()
    return _CACHE["nc"]


def make_in_maps(x, w_qkv):
    """Shard full inputs into per-core input maps. Core i = (b=i//4, h=i%4)."""
    x = np.ascontiguousarray(np.asarray(x, dtype=np.float32))
    w_qkv = np.ascontiguousarray(np.asarray(w_qkv, dtype=np.float32))
    b_, c, d, hh, ww = x.shape
    xf = x.reshape(b_, c, d * hh * ww)
    in_maps = []
    for core in range(NCORES):
        b, h = divmod(core, HEADS)
        rows = np.concatenate([
            np.arange(h * HD, (h + 1) * HD),
            np.arange(C + h * HD, C + (h + 1) * HD),
            np.arange(2 * C + h * HD, 2 * C + (h + 1) * HD),
        ])
        w_h = np.ascontiguousarray(w_qkv[rows, :])          # [48, 64]
        wT_h = np.ascontiguousarray(w_h.T)                   # [64, 48]
        x_b = np.ascontiguousarray(xf[b])                    # [64, 4096]
        x_res = np.ascontiguousarray(x_b[h * HD : (h + 1) * HD])  # [16, 4096]
        # col 0 sums q squares -> partition 0; col 32 sums k squares ->
        # partition 32 (PSUM reads must start 32-aligned)
        ones_pat = np.zeros((2 * HD, 33), dtype=np.float32)
        ones_pat[0:HD, 0] = 1.0
        ones_pat[HD : 2 * HD, 32] = 1.0
        in_maps.append({"x": x_b, "w": w_h, "wT": wT_h, "xres": x_res,
                        "onespat": ones_pat})
    return in_maps


def assemble_output(results, x_shape):
    b_, c, d, hh, ww = x_shape
    out = np.empty((b_, c, d * hh * ww), dtype=np.float32)
    for core in range(NCORES):
        b, h = divmod(core, HEADS)
        out[b, h * HD : (h + 1) * HD] = results[core]["out"]
    return out.reshape(x_shape)


def run(x, w_qkv, trace=False, **kw):
    nc = _get_program()
    in_maps = make_in_maps(x, w_qkv)
    res = run_bass_kernel_spmd(nc, in_maps, list(range(NCORES)),
                               trace=trace, **kw)
    return assemble_output(res.results, np.asarray(x).shape), res


def kernel(x, w_qkv):
    out, _ = run(x, w_qkv)
    return out
